# revision 39
# baseline (speedup 1.0000x reference)
"""Cached self-attention Trainium2 kernel (v4).

Sharding: 8 cores = 2 batches x 4 head-groups. Core c: batch b=c//4, group
g=c%4 owns heads 4g..4g+3 (columns 512g:512g+512 of the q/k/v projections).

v4 pipeline (single tensor-engine stream, no phase barriers):
  qk0 proj -> v proj (all heads) -> attn(j) with qk(j+1) projection
  matmuls interleaved as fill work -> per-(head, query-block) AllGather
  fired as soon as that block is normalized -> out-projection per
  query-block, the first block filled into head-3's attention windows.

The attention inner loop is ScalarE-exp bound; fill matmuls keep the PE
busy. The softmax z chain (ones-matmul cross-partition sum, reciprocal,
ones-matmul broadcast, normalize) for block (j, sb) is deferred into the
next block's windows so the PE never head-of-line blocks on DVE.

PSUM budget (8 banks): pss 2x[128,2,512] = 4, PA 2x[128,512] = 2,
acc1 (fill proj / psO) 1, z (psz/psb shared sequentially) 1.
"""
import numpy as np
from collections import deque
from contextlib import ExitStack

import concourse.bass as bass
import concourse.tile as tile
from concourse import bacc, mybir
from concourse.bass_utils import run_bass_kernel_spmd

B, S, PC, D, H = 2, 2048, 2048, 2048, 16
HD = D // H            # 128 head dim
GH = H // 4            # 4 heads per core
DG = GH * HD           # 512 head-dims per core
NB = 512               # query block / matmul free dim
NKC = (PC + S) // HD   # 32 key chunks of 128
NCC = PC // HD         # 16 cache chunks
NDC = D // HD          # 16 contraction chunks
F16 = mybir.dt.float16
F32 = mybir.dt.float32
AF = mybir.ActivationFunctionType
ALU = mybir.AluOpType
INV_SQRT_HD = float(1.0 / np.sqrt(HD))

GROUPS = [[0, 1, 2, 3], [4, 5, 6, 7]]


def build():
    nc = bacc.Bacc("TRN2", target_bir_lowering=False, debug=False, num_devices=8)

    def inp(name, shape, dt=F16):
        return nc.dram_tensor(name, shape, dt, kind="ExternalInput").ap()

    # all inputs host-packed so SBUF loads are fat contiguous descriptors
    xT = inp("xT", [HD, NDC, S])        # x[b].T as [p, kc, s]
    wq = inp("wq", [HD, GH, NDC, HD])   # per-head [p, kc, n], / sqrt(HD)
    bq = inp("bq", [HD, GH], F32)       # bq / sqrt(HD), [p, m]
    wk = inp("wk", [HD, GH, NDC, HD])
    bk = inp("bk", [HD, GH], F32)
    wv = inp("wv", [HD, NDC, DG])
    bv = inp("bv", [1, DG])
    ckT = inp("ckT", [HD, GH, PC])      # cache_k.T as [p, m, s]
    cv = inp("cv", [HD, NCC, DG])       # cache_v as [p, ss, d]
    wo = inp("wo", [HD, 16, NB])        # rows permuted to (4j+r), [p, c, n]
    bo = inp("bo", [1, DG])
    y = nc.dram_tensor("y", [S, DG], F32, kind="ExternalOutput").ap()

    with tile.TileContext(nc) as tc, ExitStack() as ctx:
        res = ctx.enter_context(tc.tile_pool(name="res", bufs=1))
        dram = ctx.enter_context(tc.tile_pool(name="dram", bufs=1, space="DRAM"))

        # tiny whole-kernel residents
        bq_t = res.tile([HD, GH], F32, tag="bq")
        bk_t = res.tile([HD, GH], F32, tag="bk")
        bv_t = res.tile([1, DG], F16, tag="bv")
        bo_t = res.tile([1, DG], F16, tag="bo")
        ones_k = res.tile([HD, 1], F16, tag="ones_k")
        ones_r16 = res.tile([1, HD], F16, tag="ones_r16")
        ones_r32 = res.tile([1, HD], F32, tag="ones_r32")
        nc.vector.memset(ones_k[:], 1.0)
        nc.vector.memset(ones_r16[:], 1.0)
        nc.vector.memset(ones_r32[:], 1.0)

        # collective bounce buffers, one per (head j, query block sb)
        bounce_in = {}
        bounce_out = {}
        for j in range(GH):
            for sb in range(4):
                bounce_in[j, sb] = dram.tile([HD, NB], F16, tag=f"bi{j}_{sb}",
                                             name=f"bi{j}_{sb}")
                bounce_out[j, sb] = dram.tile([4, HD, NB], F16,
                                              tag=f"bg{j}_{sb}",
                                              name=f"bg{j}_{sb}")

        # ---- long-lived SBUF pools ----
        ph = ctx.enter_context(tc.tile_pool(name="ph", bufs=1))
        qkp = ctx.enter_context(tc.tile_pool(name="qkp", bufs=2))
        wtp = ctx.enter_context(tc.tile_pool(name="wtp", bufs=2))
        e2p = ctx.enter_context(tc.tile_pool(name="e2p", bufs=3))
        zap = ctx.enter_context(tc.tile_pool(name="zap", bufs=2))
        zp = ctx.enter_context(tc.tile_pool(name="zp", bufs=2))
        ahp = ctx.enter_context(tc.tile_pool(name="ahp", bufs=3))
        # ltp / wotp created later, once earlier pools are released
        # PSUM pools (8 banks total)
        pssp = ctx.enter_context(tc.tile_pool(name="pssp", bufs=2, space="PSUM"))
        pap = ctx.enter_context(tc.tile_pool(name="pap", bufs=2, space="PSUM"))
        pacc = ctx.enter_context(tc.tile_pool(name="pacc", bufs=1, space="PSUM"))
        pz = ctx.enter_context(tc.tile_pool(name="pz", bufs=1, space="PSUM"))

        ckT_t = ph.tile([HD, GH, PC], F16, tag="ckT")
        cv_t = ph.tile([HD, NCC, DG], F16, tag="cv")
        vn_t = ph.tile([HD, S // HD, DG], F16, tag="vn")

        def load_wt(wsrc, m, name):
            wt = wtp.tile([HD, NDC, HD], F16, tag="wt", name=name)
            for q in range(4):
                nc.sync.dma_start(wt[:, 4 * q:4 * (q + 1), :],
                                  wsrc[:, m, 4 * q:4 * (q + 1), :])
            return wt

        # scoped pools closed mid-emission to recycle SBUF
        xp_es = ExitStack()
        xp = xp_es.enter_context(tc.tile_pool(name="xp", bufs=1))
        wvp_es = ExitStack()
        wvp = wvp_es.enter_context(tc.tile_pool(name="wvp", bufs=1))

        # ---- input DMAs, ordered by first consumption; big loads split
        # across several dma_starts so they spread over DMA queues ----
        wt_q0 = load_wt(wq, 0, "wt_q0")
        wt_k0 = load_wt(wk, 0, "wt_k0")
        xres = xp.tile([HD, NDC, S], F16, tag="xres")   # 8.4 MB
        for kc in range(4):   # first chunks split in half: faster first MM
            for h in range(2):
                nc.sync.dma_start(xres[:, kc, S // 2 * h:S // 2 * (h + 1)],
                                  xT[:, kc, S // 2 * h:S // 2 * (h + 1)])
        for kc in range(4, NDC):
            nc.sync.dma_start(xres[:, kc, :], xT[:, kc, :])
        wvt = wvp.tile([HD, NDC, DG], F16, tag="wvt")    # 2.1 MB
        for q in range(4):
            nc.sync.dma_start(wvt[:, 4 * q:4 * (q + 1), :],
                              wv[:, 4 * q:4 * (q + 1), :])
        nc.sync.dma_start(bq_t[:], bq)
        nc.sync.dma_start(bk_t[:], bk)
        nc.sync.dma_start(bv_t[:], bv)
        nc.sync.dma_start(bo_t[:], bo)
        for m in range(GH):
            nc.sync.dma_start(ckT_t[:, m, :], ckT[:, m, :])
        for q in range(4):
            nc.sync.dma_start(cv_t[:, 4 * q:4 * (q + 1), :],
                              cv[:, 4 * q:4 * (q + 1), :])

        qT = {}
        kT = {}

        # ---- projection emitters ----
        def proj_qk_emit(m, wt, dst, bias_t):
            """Serial q-or-k projection for head m (4 sb groups), pss slots."""
            for sb in range(4):
                ps = pssp.tile([HD, 2, NB], F32, tag="pss", name="psqk")
                for kc in range(NDC):
                    nc.tensor.matmul(ps[:, 0, :], wt[:, kc, :],
                                     xres[:, kc, NB * sb:NB * (sb + 1)],
                                     start=(kc == 0), stop=(kc == NDC - 1))
                nc.vector.tensor_scalar_add(
                    dst[:, NB * sb:NB * (sb + 1)], ps[:, 0, :],
                    bias_t[:, m:m + 1])

        def proj_qk_tasks(m, wsrc, dst, bias_t, wname):
            """Fill-task closures (2 MMs each) for q-or-k proj of head m."""
            state = {}

            def t_load():
                state["wt"] = load_wt(wsrc, m, wname)
            yield t_load
            for sb in range(4):
                def t_start(sb=sb):
                    ps = pacc.tile([HD, NB], F32, tag="acc1", name="psqk")
                    state["ps"] = ps
                    for kc in range(2):
                        nc.tensor.matmul(
                            ps[:], state["wt"][:, kc, :],
                            xres[:, kc, NB * sb:NB * (sb + 1)],
                            start=(kc == 0), stop=False,
                            skip_group_check=True)
                yield t_start
                for kc0 in range(2, NDC, 2):
                    def t_mm(sb=sb, kc0=kc0):
                        for kc in range(kc0, kc0 + 2):
                            nc.tensor.matmul(
                                state["ps"][:], state["wt"][:, kc, :],
                                xres[:, kc, NB * sb:NB * (sb + 1)],
                                start=False, stop=(kc == NDC - 1),
                                skip_group_check=True)
                    yield t_mm

                def t_evac(sb=sb):
                    nc.vector.tensor_scalar_add(
                        dst[:, NB * sb:NB * (sb + 1)], state["ps"][:],
                        bias_t[:, m:m + 1])
                yield t_evac

        # ---- head 0 q/k projection (kc-outer: chunk-paced with the x DMA,
        # 4 concurrent accumulators in the two pss slots) ----
        def proj_qk_kcouter(wt, dst, bias_t):
            acc_t = [pssp.tile([HD, 2, NB], F32, tag="pss", name="psqk0")
                     for _ in range(2)]
            accs = [acc_t[sb // 2][:, sb % 2, :] for sb in range(4)]
            for kc in range(NDC):
                for sb in range(4):
                    nc.tensor.matmul(accs[sb], wt[:, kc, :],
                                     xres[:, kc, NB * sb:NB * (sb + 1)],
                                     start=(kc == 0), stop=(kc == NDC - 1),
                                     skip_group_check=True)
            for sb in range(4):
                nc.vector.tensor_scalar_add(
                    dst[:, NB * sb:NB * (sb + 1)], accs[sb],
                    bias_t[:, 0:1])

        with nc.named_scope("seg_qk0"):
            qT[0] = qkp.tile([HD, S], F16, tag="qT", name="qT0")
            kT[0] = qkp.tile([HD, S], F16, tag="kT", name="kT0")
            proj_qk_kcouter(wt_q0, qT[0], bq_t)
            proj_qk_kcouter(wt_k0, kT[0], bk_t)

        # one-time broadcast of bv (and later bo) to all partitions, so the
        # per-ss bias matmuls become DVE adds during PSUM evacuation
        bvb = res.tile([HD, NB], F32, tag="bvb")
        bob = res.tile([HD, NB], F32, tag="bob")
        psbv = pz.tile([HD, NB], F32, tag="z", name="psbv")
        nc.tensor.matmul(psbv[:], ones_r16[:], bv_t[:], start=True, stop=True)
        nc.vector.tensor_copy(bvb[:], psbv[:])

        with nc.named_scope("seg_v"):
            for ss in range(S // HD):
                psv = pssp.tile([HD, 2, NB], F32, tag="pss", name="psv")
                for kc in range(NDC):
                    nc.tensor.matmul(psv[:, 0, :],
                                     xres[:, kc, HD * ss:HD * (ss + 1)],
                                     wvt[:, kc, :],
                                     start=(kc == 0), stop=(kc == NDC - 1))
                nc.vector.tensor_tensor(vn_t[:, ss, :], psv[:, 0, :], bvb[:],
                                        ALU.add)
        wvp_es.close()

        # ---- fill machinery ----
        fill = deque()

        def pump(n):
            for _ in range(n):
                if fill:
                    fill.popleft()()

        def drain_fill():
            while fill:
                fill.popleft()()

        # ---- deferred softmax z chain ----
        def chain_steps(j, sb, pa, zacc2):
            st = {}

            def s0():
                st["zfin"] = zp.tile([HD, NB], F16, tag="zfin", name="zfin")
                nc.vector.tensor_tensor(st["zfin"][:], zacc2[:, 0, :],
                                        zacc2[:, 1, :], ALU.add)

            def s1():
                st["psz"] = pz.tile([1, NB], F32, tag="z", name="psz")
                nc.tensor.matmul(st["psz"][:], ones_k[:], st["zfin"][:],
                                 start=True, stop=True, skip_group_check=True)

            def s2():
                zi32 = zp.tile([1, NB], F32, tag="zinv32", name="zinv32")
                st["zinv"] = zp.tile([1, NB], F16, tag="zinv", name="zinv")
                nc.vector.reciprocal_approx_fast(zi32[:], st["psz"][:])
                nc.vector.tensor_copy(st["zinv"][:], zi32[:])

            def s3():
                st["psb"] = pz.tile([HD, NB], F32, tag="z", name="psb")
                nc.tensor.matmul(st["psb"][:], ones_r16[:], st["zinv"][:],
                                 start=True, stop=True, skip_group_check=True)

            def s4():
                st["zbs"] = zp.tile([HD, NB], F32, tag="zbs", name="zbs")
                nc.vector.tensor_copy(st["zbs"][:], st["psb"][:])

            def s5():
                st["ahl"] = ahp.tile([HD, NB], F16, tag="ahl", name="ahl")
                nc.vector.tensor_tensor(st["ahl"][:], pa[:], st["zbs"][:],
                                        ALU.mult)

            def s6():
                nc.sync.dma_start(bounce_in[j, sb][:], st["ahl"][:])

            def s7():
                nc.gpsimd.collective_compute(
                    "AllGather", ALU.bypass, replica_groups=GROUPS,
                    ins=[bounce_in[j, sb].opt()],
                    outs=[bounce_out[j, sb].opt()])
            return [s0, s1, s2, s3, s4, s5, s6, s7]

        pending_chain = deque()

        def flush_chain():
            while pending_chain:
                pending_chain.popleft()()

        # ---- attention block ----
        def attn_block(j, sb):
            """One (head, query-block): 16 windows of 2 key chunks each.

            Per window: one deferred-z-chain step (or a fill task), plus
            fill tasks paced so the queue drains by the end of this head.
            """
            pa = pap.tile([HD, NB], F32, tag="PA", name="PA")
            zacc2 = zap.tile([HD, 2, NB], F16, tag="zacc2", name="zacc2")
            qTs = qT[j][:, NB * sb:NB * (sb + 1)]

            def av_and_zacc(e2, c2):
                """AV matmuls + z accumulation for window c2 (lagged)."""
                for i in range(2):
                    c = 2 * c2 + i
                    if c < NCC:
                        vt = cv_t[:, c, HD * j:HD * (j + 1)]
                    else:
                        vt = vn_t[:, c - NCC, HD * j:HD * (j + 1)]
                    nc.tensor.matmul(pa[:], vt, e2[:, i, :],
                                     start=(c == 0), stop=(c == NKC - 1),
                                     skip_group_check=True)
                if c2 == 0:
                    nc.vector.tensor_copy(zacc2[:], e2[:])
                else:
                    nc.vector.tensor_tensor(zacc2[:], zacc2[:], e2[:],
                                            ALU.add)

            prev_e = None
            for c2 in range(NKC // 2):
                pss = pssp.tile([HD, 2, NB], F32, tag="pss", name="pss")
                e2 = e2p.tile([HD, 2, NB], F16, tag="e", name="e2")
                for i in range(2):
                    c = 2 * c2 + i
                    if c < NCC:
                        kt = ckT_t[:, j, HD * c:HD * (c + 1)]
                    else:
                        kt = kT[j][:, HD * (c - NCC):HD * (c - NCC + 1)]
                    nc.tensor.matmul(pss[:, i, :], kt, qTs,
                                     start=True, stop=True)
                nc.scalar.activation(e2[:], pss[:], AF.Exp)
                if prev_e is not None:
                    av_and_zacc(prev_e, c2 - 1)
                prev_e = e2
                for _ in range(2):
                    if pending_chain:
                        pending_chain.popleft()()
                    else:
                        pump(1)
                # pace remaining fill over the windows left in this head
                wleft = (3 - sb) * 16 + (15 - c2)
                if len(fill) > wleft:
                    pump(2)
                elif len(fill) > wleft // 2:
                    pump(1)
            av_and_zacc(prev_e, NKC // 2 - 1)
            return pa, zacc2

        # ---- out-projection helpers ----
        lt = {}
        wot_box = {}

        def load_lt(j, sb):
            t = ltp.tile([HD, 4, NB], F16, tag=f"lt{j}", name=f"lt{j}_{sb}")
            lt[j, sb] = t
            for r in range(4):
                for h in range(2):
                    nc.sync.dma_start(
                        t[:, r, NB // 2 * h:NB // 2 * (h + 1)],
                        bounce_out[j, sb][r][:, NB // 2 * h:NB // 2 * (h + 1)])

        po = {}

        def psO_pass1_tasks(m):
            """Out-proj chunk m, heads j=0..2 (their gathers land early);
            partial (+bo broadcast) parked in SBUF."""
            sb, mm = divmod(m, 4)
            st = {}

            def t_start():
                ps = pacc.tile([HD, NB], F32, tag="acc1", name="psO1")
                st["ps"] = ps[:]
                nc.tensor.matmul(
                    st["ps"], lt[0, sb][:, 0, HD * mm:HD * (mm + 1)],
                    wot_box["wot"][:, 0, :],
                    start=True, stop=False, skip_group_check=True)
            yield t_start
            for jr0 in range(1, 12, 2):
                def t_mm(jr0=jr0):
                    for jr in range(jr0, min(jr0 + 2, 12)):
                        jj, r = divmod(jr, 4)
                        nc.tensor.matmul(
                            st["ps"], lt[jj, sb][:, r, HD * mm:HD * (mm + 1)],
                            wot_box["wot"][:, jr, :],
                            start=False, stop=(jr == 11),
                            skip_group_check=True)
                yield t_mm

            def t_evac(m=m):
                p = pop.tile([HD, NB], F32, tag="po", name=f"po{m}")
                po[m] = p
                nc.vector.tensor_tensor(p[:], st["ps"], bob[:], ALU.add)
            yield t_evac

        def psO_pass2_tasks(m):
            """Out-proj chunk m, head j=3, combined with the pass-1 partial."""
            sb, mm = divmod(m, 4)
            st = {}

            def t_start():
                ps = pacc.tile([HD, NB], F32, tag="acc1", name="psO2")
                st["ps"] = ps[:]
                for r in range(2):
                    nc.tensor.matmul(
                        st["ps"], lt[3, sb][:, r, HD * mm:HD * (mm + 1)],
                        wot_box["wot"][:, 12 + r, :],
                        start=(r == 0), stop=False, skip_group_check=True)
            yield t_start

            def t_fin():
                for r in range(2, 4):
                    nc.tensor.matmul(
                        st["ps"], lt[3, sb][:, r, HD * mm:HD * (mm + 1)],
                        wot_box["wot"][:, 12 + r, :],
                        start=False, stop=(r == 3), skip_group_check=True)
            yield t_fin

            def t_evac(m=m):
                ot = ahp.tile([HD, NB], F32, tag="ot", name="ot")
                nc.vector.tensor_tensor(ot[:], st["ps"], po[m][:], ALU.add)
                nc.sync.dma_start(y[HD * m:HD * (m + 1), :], ot[:])
            yield t_evac

        # ================= emission schedule =================
        prev = None   # (j, sb, pa, zacc2) of the previous attention block
        for j in range(GH):
            if j + 1 < GH:
                qT[j + 1] = qkp.tile([HD, S], F16, tag="qT", name=f"qT{j+1}")
                kT[j + 1] = qkp.tile([HD, S], F16, tag="kT", name=f"kT{j+1}")
                fill.extend(proj_qk_tasks(j + 1, wq, qT[j + 1], bq_t,
                                          f"wt_q{j+1}"))
                fill.extend(proj_qk_tasks(j + 1, wk, kT[j + 1], bk_t,
                                          f"wt_k{j+1}"))
            if j == GH - 1:
                # all projections emitted; free xres for reuse
                drain_fill()
                xp_es.close()
                ltp = ctx.enter_context(tc.tile_pool(name="ltp", bufs=2))
                wotp = ctx.enter_context(tc.tile_pool(name="wotp", bufs=1))
                pop = ctx.enter_context(tc.tile_pool(name="pop", bufs=16))
                wot_box["wot"] = wotp.tile([HD, 16, NB], F16, tag="wot",
                                           name="wot")
                for q in range(4):
                    nc.sync.dma_start(
                        wot_box["wot"][:, 4 * q:4 * (q + 1), :],
                        wo[:, 4 * q:4 * (q + 1), :])
                # one-time broadcast of bo for the out-proj partial adds
                psbo = pz.tile([HD, NB], F32, tag="z", name="psbo")
                nc.tensor.matmul(psbo[:], ones_r16[:], bo_t[:],
                                 start=True, stop=True, skip_group_check=True)
                nc.vector.tensor_copy(bob[:], psbo[:])
            with nc.named_scope(f"attn{j}"):
                for sb in range(4):
                    if prev is not None:
                        pending_chain.extend(chain_steps(prev[0], prev[1],
                                                         prev[2], prev[3]))
                    if j == GH - 1:
                        # pass 1 of out-proj block sb (heads 0..2 gathered
                        # long ago) fills this attention block; pass 2 of
                        # block sb-2 follows once gather (3, sb-2) landed.
                        # sb3's pass 1 is held back to overlap the final
                        # gather in the tail.
                        for jj in range(3):
                            load_lt(jj, sb)
                        if sb < 3:
                            for m in range(4 * sb, 4 * sb + 4):
                                fill.extend(psO_pass1_tasks(m))
                        if sb >= 2:
                            load_lt(3, sb - 2)
                            for m in range(4 * (sb - 2), 4 * (sb - 2) + 4):
                                fill.extend(psO_pass2_tasks(m))
                    pa, zacc2 = attn_block(j, sb)
                    prev = (j, sb, pa, zacc2)

        # ---- tail: z chain for (3,3), remaining out-projection ----
        with nc.named_scope("tail"):
            flush_chain()
            pending_chain.extend(chain_steps(prev[0], prev[1],
                                             prev[2], prev[3]))
            flush_chain()
            drain_fill()
            # pass 1 of sb3 and pass 2 of sb2 overlap gather (3,3)'s flight
            for m in range(12, 16):
                for t in psO_pass1_tasks(m):
                    t()
            for sb in (2, 3):
                load_lt(3, sb)
                for m in range(4 * sb, 4 * sb + 4):
                    for t in psO_pass2_tasks(m):
                        t()

    nc.compile()
    return nc


_BUILT = None


def get_built():
    global _BUILT
    if _BUILT is None:
        _BUILT = build()
    return _BUILT


def make_in_maps(x, cache_k, cache_v, wq, bq, wk, bk, wv, bv, wo, bo):
    x = np.asarray(x)
    cache_k = np.asarray(cache_k)
    cache_v = np.asarray(cache_v)
    wq, bq = np.asarray(wq), np.asarray(bq)
    wk, bk = np.asarray(wk), np.asarray(bk)
    wv, bv = np.asarray(wv), np.asarray(bv)
    wo, bo = np.asarray(wo), np.asarray(bo)

    # permute wo rows to match gather order: chunk jr=(4j+r) holds head 4r+j
    perm = np.concatenate([
        np.arange(HD * (4 * r + j), HD * (4 * r + j) + HD)
        for j in range(GH) for r in range(4)
    ])
    wo_p = wo[perm, :]

    def pack_w(w):
        """[D, cols] -> [HD(p), D//(HD), cols]: row kc*HD+p at [p, kc]."""
        return np.ascontiguousarray(
            w.reshape(-1, HD, w.shape[1]).transpose(1, 0, 2))

    def pack_w_heads(w):
        """[D, DG] -> [HD(p), GH(m), NDC(kc), HD(n)]."""
        return np.ascontiguousarray(
            w.reshape(NDC, HD, GH, HD).transpose(1, 2, 0, 3))

    in_maps = []
    for c in range(8):
        b, g = divmod(c, 4)
        sl = slice(DG * g, DG * (g + 1))
        xTb = x[b].T
        in_maps.append({
            "xT": pack_w(xTb).astype(np.float16),
            "wq": pack_w_heads(wq[:, sl] * INV_SQRT_HD).astype(np.float16),
            "bq": np.ascontiguousarray(
                (bq[sl] * INV_SQRT_HD).reshape(GH, HD).T).astype(np.float32),
            "wk": pack_w_heads(wk[:, sl]).astype(np.float16),
            "bk": np.ascontiguousarray(
                bk[sl].reshape(GH, HD).T).astype(np.float32),
            "wv": pack_w(wv[:, sl]).astype(np.float16),
            "bv": bv[sl][None, :].astype(np.float16),
            "ckT": pack_w(cache_k[b][:, sl].T).astype(np.float16),
            "cv": pack_w(cache_v[b][:, sl]).astype(np.float16),
            "wo": pack_w(wo_p[:, sl]).astype(np.float16),
            "bo": bo[sl][None, :].astype(np.float16),
        })
    return in_maps


def assemble(results):
    out = np.empty((B, S, D), np.float32)
    for c in range(8):
        b, g = divmod(c, 4)
        out[b, :, DG * g:DG * (g + 1)] = results[c]["y"]
    return out


def kernel(**inputs):
    nc = get_built()
    in_maps = make_in_maps(**inputs)
    res = run_bass_kernel_spmd(nc, in_maps, core_ids=list(range(8)))
    return assemble(res.results)


# revision 40
# speedup vs baseline: 1.0138x; 1.0138x over previous
"""Cached self-attention Trainium2 kernel (v4).

Sharding: 8 cores = 2 batches x 4 head-groups. Core c: batch b=c//4, group
g=c%4 owns heads 4g..4g+3 (columns 512g:512g+512 of the q/k/v projections).

v4 pipeline (single tensor-engine stream, no phase barriers):
  qk0 proj -> v proj (all heads) -> attn(j) with qk(j+1) projection
  matmuls interleaved as fill work -> per-(head, query-block) AllGather
  fired as soon as that block is normalized -> out-projection per
  query-block, the first block filled into head-3's attention windows.

The attention inner loop is ScalarE-exp bound; fill matmuls keep the PE
busy. The softmax z chain (ones-matmul cross-partition sum, reciprocal,
ones-matmul broadcast, normalize) for block (j, sb) is deferred into the
next block's windows so the PE never head-of-line blocks on DVE.

PSUM budget (8 banks): pss 2x[128,2,512] = 4, PA 2x[128,512] = 2,
acc1 (fill proj / psO) 1, z (psz/psb shared sequentially) 1.
"""
import numpy as np
from collections import deque
from contextlib import ExitStack

import concourse.bass as bass
import concourse.tile as tile
from concourse import bacc, mybir
from concourse.bass_utils import run_bass_kernel_spmd

B, S, PC, D, H = 2, 2048, 2048, 2048, 16
HD = D // H            # 128 head dim
GH = H // 4            # 4 heads per core
DG = GH * HD           # 512 head-dims per core
NB = 512               # query block / matmul free dim
NKC = (PC + S) // HD   # 32 key chunks of 128
NCC = PC // HD         # 16 cache chunks
NDC = D // HD          # 16 contraction chunks
F16 = mybir.dt.float16
F32 = mybir.dt.float32
AF = mybir.ActivationFunctionType
ALU = mybir.AluOpType
INV_SQRT_HD = float(1.0 / np.sqrt(HD))

GROUPS = [[0, 1, 2, 3], [4, 5, 6, 7]]


def build():
    nc = bacc.Bacc("TRN2", target_bir_lowering=False, debug=False, num_devices=8)

    def inp(name, shape, dt=F16):
        return nc.dram_tensor(name, shape, dt, kind="ExternalInput").ap()

    # all inputs host-packed so SBUF loads are fat contiguous descriptors
    xT = inp("xT", [HD, NDC, S])        # x[b].T as [p, kc, s]
    wq = inp("wq", [HD, GH, NDC, HD])   # per-head [p, kc, n], / sqrt(HD)
    bq = inp("bq", [HD, GH], F32)       # bq / sqrt(HD), [p, m]
    wk = inp("wk", [HD, GH, NDC, HD])
    bk = inp("bk", [HD, GH], F32)
    wv = inp("wv", [HD, NDC, DG])
    bv = inp("bv", [1, DG])
    ckT = inp("ckT", [HD, GH, PC])      # cache_k.T as [p, m, s]
    cv = inp("cv", [HD, NCC, DG])       # cache_v as [p, ss, d]
    wo = inp("wo", [HD, 16, NB])        # rows permuted to (4j+r), [p, c, n]
    bo = inp("bo", [1, DG])
    y = nc.dram_tensor("y", [S, DG], F32, kind="ExternalOutput").ap()

    with tile.TileContext(nc) as tc, ExitStack() as ctx:
        res = ctx.enter_context(tc.tile_pool(name="res", bufs=1))
        dram = ctx.enter_context(tc.tile_pool(name="dram", bufs=1, space="DRAM"))

        # tiny whole-kernel residents
        bq_t = res.tile([HD, GH], F32, tag="bq")
        bk_t = res.tile([HD, GH], F32, tag="bk")
        bv_t = res.tile([1, DG], F16, tag="bv")
        bo_t = res.tile([1, DG], F16, tag="bo")
        ones_k = res.tile([HD, 1], F16, tag="ones_k")
        ones_r16 = res.tile([1, HD], F16, tag="ones_r16")
        ones_r32 = res.tile([1, HD], F32, tag="ones_r32")
        nc.vector.memset(ones_k[:], 1.0)
        nc.vector.memset(ones_r16[:], 1.0)
        nc.vector.memset(ones_r32[:], 1.0)

        # collective bounce buffers, one per (head j, query block sb)
        bounce_in = {}
        bounce_out = {}
        for j in range(GH):
            for sb in range(4):
                bounce_in[j, sb] = dram.tile([HD, NB], F16, tag=f"bi{j}_{sb}",
                                             name=f"bi{j}_{sb}")
                bounce_out[j, sb] = dram.tile([4, HD, NB], F16,
                                              tag=f"bg{j}_{sb}",
                                              name=f"bg{j}_{sb}")

        # ---- long-lived SBUF pools ----
        ph = ctx.enter_context(tc.tile_pool(name="ph", bufs=1))
        qkp = ctx.enter_context(tc.tile_pool(name="qkp", bufs=2))
        wtp = ctx.enter_context(tc.tile_pool(name="wtp", bufs=2))
        e2p = ctx.enter_context(tc.tile_pool(name="e2p", bufs=3))
        zap = ctx.enter_context(tc.tile_pool(name="zap", bufs=2))
        zp = ctx.enter_context(tc.tile_pool(name="zp", bufs=2))
        ahp = ctx.enter_context(tc.tile_pool(name="ahp", bufs=3))
        # ltp / wotp created later, once earlier pools are released
        # PSUM pools (8 banks total)
        pssp = ctx.enter_context(tc.tile_pool(name="pssp", bufs=2, space="PSUM"))
        pap = ctx.enter_context(tc.tile_pool(name="pap", bufs=2, space="PSUM"))
        pacc = ctx.enter_context(tc.tile_pool(name="pacc", bufs=1, space="PSUM"))
        pz = ctx.enter_context(tc.tile_pool(name="pz", bufs=1, space="PSUM"))

        ckT_t = ph.tile([HD, GH, PC], F16, tag="ckT")
        cv_t = ph.tile([HD, NCC, DG], F16, tag="cv")
        vn_t = ph.tile([HD, S // HD, DG], F16, tag="vn")

        def load_wt(wsrc, m, name):
            wt = wtp.tile([HD, NDC, HD], F16, tag="wt", name=name)
            for q in range(4):
                nc.sync.dma_start(wt[:, 4 * q:4 * (q + 1), :],
                                  wsrc[:, m, 4 * q:4 * (q + 1), :])
            return wt

        # scoped pools closed mid-emission to recycle SBUF
        xp_es = ExitStack()
        xp = xp_es.enter_context(tc.tile_pool(name="xp", bufs=1))
        wvp_es = ExitStack()
        wvp = wvp_es.enter_context(tc.tile_pool(name="wvp", bufs=1))

        # ---- input DMAs, ordered by first consumption; big loads split
        # across several dma_starts so they spread over DMA queues ----
        wt_q0 = load_wt(wq, 0, "wt_q0")
        wt_k0 = load_wt(wk, 0, "wt_k0")
        xres = xp.tile([HD, NDC, S], F16, tag="xres")   # 8.4 MB
        for kc in range(4):   # first chunks split in half: faster first MM
            for h in range(2):
                nc.sync.dma_start(xres[:, kc, S // 2 * h:S // 2 * (h + 1)],
                                  xT[:, kc, S // 2 * h:S // 2 * (h + 1)])
        for kc in range(4, NDC):
            nc.sync.dma_start(xres[:, kc, :], xT[:, kc, :])
        wvt = wvp.tile([HD, NDC, DG], F16, tag="wvt")    # 2.1 MB
        for q in range(4):
            nc.sync.dma_start(wvt[:, 4 * q:4 * (q + 1), :],
                              wv[:, 4 * q:4 * (q + 1), :])
        nc.sync.dma_start(bq_t[:], bq)
        nc.sync.dma_start(bk_t[:], bk)
        nc.sync.dma_start(bv_t[:], bv)
        nc.sync.dma_start(bo_t[:], bo)
        for m in range(GH):
            nc.sync.dma_start(ckT_t[:, m, :], ckT[:, m, :])
        for q in range(4):
            nc.sync.dma_start(cv_t[:, 4 * q:4 * (q + 1), :],
                              cv[:, 4 * q:4 * (q + 1), :])

        qT = {}
        kT = {}

        # ---- projection emitters ----
        def proj_qk_emit(m, wt, dst, bias_t):
            """Serial q-or-k projection for head m (4 sb groups), pss slots."""
            for sb in range(4):
                ps = pssp.tile([HD, 2, NB], F32, tag="pss", name="psqk")
                for kc in range(NDC):
                    nc.tensor.matmul(ps[:, 0, :], wt[:, kc, :],
                                     xres[:, kc, NB * sb:NB * (sb + 1)],
                                     start=(kc == 0), stop=(kc == NDC - 1))
                nc.vector.tensor_scalar_add(
                    dst[:, NB * sb:NB * (sb + 1)], ps[:, 0, :],
                    bias_t[:, m:m + 1])

        def proj_qk_tasks(m, wsrc, dst, bias_t, wname):
            """Fill-task closures (2 MMs each) for q-or-k proj of head m."""
            state = {}

            def t_load():
                state["wt"] = load_wt(wsrc, m, wname)
            yield t_load
            for sb in range(4):
                def t_start(sb=sb):
                    ps = pacc.tile([HD, NB], F32, tag="acc1", name="psqk")
                    state["ps"] = ps
                    for kc in range(2):
                        nc.tensor.matmul(
                            ps[:], state["wt"][:, kc, :],
                            xres[:, kc, NB * sb:NB * (sb + 1)],
                            start=(kc == 0), stop=False,
                            skip_group_check=True)
                yield t_start
                for kc0 in range(2, NDC, 2):
                    def t_mm(sb=sb, kc0=kc0):
                        for kc in range(kc0, kc0 + 2):
                            nc.tensor.matmul(
                                state["ps"][:], state["wt"][:, kc, :],
                                xres[:, kc, NB * sb:NB * (sb + 1)],
                                start=False, stop=(kc == NDC - 1),
                                skip_group_check=True)
                    yield t_mm

                def t_evac(sb=sb):
                    nc.vector.tensor_scalar_add(
                        dst[:, NB * sb:NB * (sb + 1)], state["ps"][:],
                        bias_t[:, m:m + 1])
                yield t_evac

        # ---- head 0 q AND k projection together, kc-outer: paced with the
        # x DMA so both finish as the last chunk lands. All 8 PSUM banks
        # are free at startup -> 8 concurrent accumulators.
        with nc.named_scope("seg_qk0"):
            qT[0] = qkp.tile([HD, S], F16, tag="qT", name="qT0")
            kT[0] = qkp.tile([HD, S], F16, tag="kT", name="kT0")
            qacc_t = [pssp.tile([HD, 2, NB], F32, tag="pss", name="q0acc")
                      for _ in range(2)]
            qaccs = [qacc_t[sb // 2][:, sb % 2, :] for sb in range(4)]
            kacc_pa = [pap.tile([HD, NB], F32, tag="PA", name="k0acc")
                       for _ in range(2)]
            kaccs = [kacc_pa[0][:], kacc_pa[1][:],
                     pacc.tile([HD, NB], F32, tag="acc1", name="k0acc")[:],
                     pz.tile([HD, NB], F32, tag="z", name="k0acc")[:]]
            for kc in range(NDC):
                for sb in range(4):
                    nc.tensor.matmul(qaccs[sb], wt_q0[:, kc, :],
                                     xres[:, kc, NB * sb:NB * (sb + 1)],
                                     start=(kc == 0), stop=(kc == NDC - 1),
                                     skip_group_check=True)
                for sb in range(4):
                    nc.tensor.matmul(kaccs[sb], wt_k0[:, kc, :],
                                     xres[:, kc, NB * sb:NB * (sb + 1)],
                                     start=(kc == 0), stop=(kc == NDC - 1),
                                     skip_group_check=True)
            for sb in range(4):
                nc.vector.tensor_scalar_add(
                    qT[0][:, NB * sb:NB * (sb + 1)], qaccs[sb], bq_t[:, 0:1])
                nc.vector.tensor_scalar_add(
                    kT[0][:, NB * sb:NB * (sb + 1)], kaccs[sb], bk_t[:, 0:1])

        # one-time broadcast of bv (and later bo) to all partitions, so the
        # per-ss bias matmuls become DVE adds during PSUM evacuation
        bvb = res.tile([HD, NB], F32, tag="bvb")
        bob = res.tile([HD, NB], F32, tag="bob")
        psbv = pz.tile([HD, NB], F32, tag="z", name="psbv")
        nc.tensor.matmul(psbv[:], ones_r16[:], bv_t[:], start=True, stop=True)
        nc.vector.tensor_copy(bvb[:], psbv[:])

        with nc.named_scope("seg_v"):
            for ss in range(S // HD):
                psv = pssp.tile([HD, 2, NB], F32, tag="pss", name="psv")
                for kc in range(NDC):
                    nc.tensor.matmul(psv[:, 0, :],
                                     xres[:, kc, HD * ss:HD * (ss + 1)],
                                     wvt[:, kc, :],
                                     start=(kc == 0), stop=(kc == NDC - 1))
                nc.vector.tensor_tensor(vn_t[:, ss, :], psv[:, 0, :], bvb[:],
                                        ALU.add)
        wvp_es.close()

        # ---- fill machinery ----
        fill = deque()

        def pump(n):
            for _ in range(n):
                if fill:
                    fill.popleft()()

        def drain_fill():
            while fill:
                fill.popleft()()

        # ---- deferred softmax z chain ----
        def chain_steps(j, sb, pa, zacc2):
            st = {}

            def s0():
                st["zfin"] = zp.tile([HD, NB], F16, tag="zfin", name="zfin")
                nc.vector.tensor_tensor(st["zfin"][:], zacc2[:, 0, :],
                                        zacc2[:, 1, :], ALU.add)

            def s1():
                st["psz"] = pz.tile([1, NB], F32, tag="z", name="psz")
                nc.tensor.matmul(st["psz"][:], ones_k[:], st["zfin"][:],
                                 start=True, stop=True, skip_group_check=True)

            def s2():
                zi32 = zp.tile([1, NB], F32, tag="zinv32", name="zinv32")
                st["zinv"] = zp.tile([1, NB], F16, tag="zinv", name="zinv")
                nc.vector.reciprocal_approx_fast(zi32[:], st["psz"][:])
                nc.vector.tensor_copy(st["zinv"][:], zi32[:])

            def s3():
                st["psb"] = pz.tile([HD, NB], F32, tag="z", name="psb")
                nc.tensor.matmul(st["psb"][:], ones_r16[:], st["zinv"][:],
                                 start=True, stop=True, skip_group_check=True)

            def s4():
                st["zbs"] = zp.tile([HD, NB], F32, tag="zbs", name="zbs")
                nc.vector.tensor_copy(st["zbs"][:], st["psb"][:])

            def s5():
                st["ahl"] = ahp.tile([HD, NB], F16, tag="ahl", name="ahl")
                nc.vector.tensor_tensor(st["ahl"][:], pa[:], st["zbs"][:],
                                        ALU.mult)

            def s6():
                nc.sync.dma_start(bounce_in[j, sb][:], st["ahl"][:])

            def s7():
                nc.gpsimd.collective_compute(
                    "AllGather", ALU.bypass, replica_groups=GROUPS,
                    ins=[bounce_in[j, sb].opt()],
                    outs=[bounce_out[j, sb].opt()])
            return [s0, s1, s2, s3, s4, s5, s6, s7]

        pending_chain = deque()

        def flush_chain():
            while pending_chain:
                pending_chain.popleft()()

        # ---- attention block ----
        def attn_block(j, sb):
            """One (head, query-block): 16 windows of 2 key chunks each.

            Per window: one deferred-z-chain step (or a fill task), plus
            fill tasks paced so the queue drains by the end of this head.
            """
            pa = pap.tile([HD, NB], F32, tag="PA", name="PA")
            zacc2 = zap.tile([HD, 2, NB], F16, tag="zacc2", name="zacc2")
            qTs = qT[j][:, NB * sb:NB * (sb + 1)]

            def av_and_zacc(e2, c2):
                """AV matmuls + z accumulation for window c2 (lagged)."""
                for i in range(2):
                    c = 2 * c2 + i
                    if c < NCC:
                        vt = cv_t[:, c, HD * j:HD * (j + 1)]
                    else:
                        vt = vn_t[:, c - NCC, HD * j:HD * (j + 1)]
                    nc.tensor.matmul(pa[:], vt, e2[:, i, :],
                                     start=(c == 0), stop=(c == NKC - 1),
                                     skip_group_check=True)
                if c2 == 0:
                    nc.vector.tensor_copy(zacc2[:], e2[:])
                else:
                    nc.vector.tensor_tensor(zacc2[:], zacc2[:], e2[:],
                                            ALU.add)

            prev_e = None
            for c2 in range(NKC // 2):
                pss = pssp.tile([HD, 2, NB], F32, tag="pss", name="pss")
                e2 = e2p.tile([HD, 2, NB], F16, tag="e", name="e2")
                for i in range(2):
                    c = 2 * c2 + i
                    if c < NCC:
                        kt = ckT_t[:, j, HD * c:HD * (c + 1)]
                    else:
                        kt = kT[j][:, HD * (c - NCC):HD * (c - NCC + 1)]
                    nc.tensor.matmul(pss[:, i, :], kt, qTs,
                                     start=True, stop=True)
                nc.scalar.activation(e2[:], pss[:], AF.Exp)
                if prev_e is not None:
                    av_and_zacc(prev_e, c2 - 1)
                prev_e = e2
                for _ in range(2):
                    if pending_chain:
                        pending_chain.popleft()()
                    else:
                        pump(1)
                # pace remaining fill over the windows left in this head
                wleft = (3 - sb) * 16 + (15 - c2)
                if len(fill) > wleft:
                    pump(2)
                elif len(fill) > wleft // 2:
                    pump(1)
            av_and_zacc(prev_e, NKC // 2 - 1)
            return pa, zacc2

        # ---- out-projection helpers ----
        lt = {}
        wot_box = {}

        def load_lt(j, sb):
            t = ltp.tile([HD, 4, NB], F16, tag=f"lt{j}", name=f"lt{j}_{sb}")
            lt[j, sb] = t
            for r in range(4):
                for h in range(2):
                    nc.sync.dma_start(
                        t[:, r, NB // 2 * h:NB // 2 * (h + 1)],
                        bounce_out[j, sb][r][:, NB // 2 * h:NB // 2 * (h + 1)])

        po = {}

        def psO_pass1_tasks(m):
            """Out-proj chunk m, heads j=0..2 (their gathers land early);
            partial (+bo broadcast) parked in SBUF."""
            sb, mm = divmod(m, 4)
            st = {}

            def t_start():
                ps = pacc.tile([HD, NB], F32, tag="acc1", name="psO1")
                st["ps"] = ps[:]
                nc.tensor.matmul(
                    st["ps"], lt[0, sb][:, 0, HD * mm:HD * (mm + 1)],
                    wot_box["wot"][:, 0, :],
                    start=True, stop=False, skip_group_check=True)
            yield t_start
            for jr0 in range(1, 12, 2):
                def t_mm(jr0=jr0):
                    for jr in range(jr0, min(jr0 + 2, 12)):
                        jj, r = divmod(jr, 4)
                        nc.tensor.matmul(
                            st["ps"], lt[jj, sb][:, r, HD * mm:HD * (mm + 1)],
                            wot_box["wot"][:, jr, :],
                            start=False, stop=(jr == 11),
                            skip_group_check=True)
                yield t_mm

            def t_evac(m=m):
                p = pop.tile([HD, NB], F32, tag="po", name=f"po{m}")
                po[m] = p
                nc.vector.tensor_tensor(p[:], st["ps"], bob[:], ALU.add)
            yield t_evac

        def psO_pass2_tasks(m):
            """Out-proj chunk m, head j=3, combined with the pass-1 partial."""
            sb, mm = divmod(m, 4)
            st = {}

            def t_start():
                ps = pacc.tile([HD, NB], F32, tag="acc1", name="psO2")
                st["ps"] = ps[:]
                for r in range(2):
                    nc.tensor.matmul(
                        st["ps"], lt[3, sb][:, r, HD * mm:HD * (mm + 1)],
                        wot_box["wot"][:, 12 + r, :],
                        start=(r == 0), stop=False, skip_group_check=True)
            yield t_start

            def t_fin():
                for r in range(2, 4):
                    nc.tensor.matmul(
                        st["ps"], lt[3, sb][:, r, HD * mm:HD * (mm + 1)],
                        wot_box["wot"][:, 12 + r, :],
                        start=False, stop=(r == 3), skip_group_check=True)
            yield t_fin

            def t_evac(m=m):
                ot = ahp.tile([HD, NB], F32, tag="ot", name="ot")
                nc.vector.tensor_tensor(ot[:], st["ps"], po[m][:], ALU.add)
                nc.sync.dma_start(y[HD * m:HD * (m + 1), :], ot[:])
            yield t_evac

        # ================= emission schedule =================
        prev = None   # (j, sb, pa, zacc2) of the previous attention block
        for j in range(GH):
            if j + 1 < GH:
                qT[j + 1] = qkp.tile([HD, S], F16, tag="qT", name=f"qT{j+1}")
                kT[j + 1] = qkp.tile([HD, S], F16, tag="kT", name=f"kT{j+1}")
                fill.extend(proj_qk_tasks(j + 1, wq, qT[j + 1], bq_t,
                                          f"wt_q{j+1}"))
                fill.extend(proj_qk_tasks(j + 1, wk, kT[j + 1], bk_t,
                                          f"wt_k{j+1}"))
            if j == GH - 1:
                # all projections emitted; free xres for reuse
                drain_fill()
                xp_es.close()
                ltp = ctx.enter_context(tc.tile_pool(name="ltp", bufs=2))
                wotp = ctx.enter_context(tc.tile_pool(name="wotp", bufs=1))
                pop = ctx.enter_context(tc.tile_pool(name="pop", bufs=16))
                wot_box["wot"] = wotp.tile([HD, 16, NB], F16, tag="wot",
                                           name="wot")
                for q in range(4):
                    nc.sync.dma_start(
                        wot_box["wot"][:, 4 * q:4 * (q + 1), :],
                        wo[:, 4 * q:4 * (q + 1), :])
                # one-time broadcast of bo for the out-proj partial adds
                psbo = pz.tile([HD, NB], F32, tag="z", name="psbo")
                nc.tensor.matmul(psbo[:], ones_r16[:], bo_t[:],
                                 start=True, stop=True, skip_group_check=True)
                nc.vector.tensor_copy(bob[:], psbo[:])
            with nc.named_scope(f"attn{j}"):
                for sb in range(4):
                    if prev is not None:
                        pending_chain.extend(chain_steps(prev[0], prev[1],
                                                         prev[2], prev[3]))
                    if j == GH - 1:
                        # pass 1 of out-proj block sb (heads 0..2 gathered
                        # long ago) fills this attention block; pass 2 of
                        # block sb-2 follows once gather (3, sb-2) landed.
                        # sb3's pass 1 is held back to overlap the final
                        # gather in the tail.
                        for jj in range(3):
                            load_lt(jj, sb)
                        if sb < 3:
                            for m in range(4 * sb, 4 * sb + 4):
                                fill.extend(psO_pass1_tasks(m))
                        if sb >= 2:
                            load_lt(3, sb - 2)
                            for m in range(4 * (sb - 2), 4 * (sb - 2) + 4):
                                fill.extend(psO_pass2_tasks(m))
                    pa, zacc2 = attn_block(j, sb)
                    prev = (j, sb, pa, zacc2)

        # ---- tail: z chain for (3,3), remaining out-projection ----
        with nc.named_scope("tail"):
            flush_chain()
            pending_chain.extend(chain_steps(prev[0], prev[1],
                                             prev[2], prev[3]))
            flush_chain()
            drain_fill()
            # pass 1 of sb3 and pass 2 of sb2 overlap gather (3,3)'s flight
            for m in range(12, 16):
                for t in psO_pass1_tasks(m):
                    t()
            for sb in (2, 3):
                load_lt(3, sb)
                for m in range(4 * sb, 4 * sb + 4):
                    for t in psO_pass2_tasks(m):
                        t()

    nc.compile()
    return nc


_BUILT = None


def get_built():
    global _BUILT
    if _BUILT is None:
        _BUILT = build()
    return _BUILT


def make_in_maps(x, cache_k, cache_v, wq, bq, wk, bk, wv, bv, wo, bo):
    x = np.asarray(x)
    cache_k = np.asarray(cache_k)
    cache_v = np.asarray(cache_v)
    wq, bq = np.asarray(wq), np.asarray(bq)
    wk, bk = np.asarray(wk), np.asarray(bk)
    wv, bv = np.asarray(wv), np.asarray(bv)
    wo, bo = np.asarray(wo), np.asarray(bo)

    # permute wo rows to match gather order: chunk jr=(4j+r) holds head 4r+j
    perm = np.concatenate([
        np.arange(HD * (4 * r + j), HD * (4 * r + j) + HD)
        for j in range(GH) for r in range(4)
    ])
    wo_p = wo[perm, :]

    def pack_w(w):
        """[D, cols] -> [HD(p), D//(HD), cols]: row kc*HD+p at [p, kc]."""
        return np.ascontiguousarray(
            w.reshape(-1, HD, w.shape[1]).transpose(1, 0, 2))

    def pack_w_heads(w):
        """[D, DG] -> [HD(p), GH(m), NDC(kc), HD(n)]."""
        return np.ascontiguousarray(
            w.reshape(NDC, HD, GH, HD).transpose(1, 2, 0, 3))

    in_maps = []
    for c in range(8):
        b, g = divmod(c, 4)
        sl = slice(DG * g, DG * (g + 1))
        xTb = x[b].T
        in_maps.append({
            "xT": pack_w(xTb).astype(np.float16),
            "wq": pack_w_heads(wq[:, sl] * INV_SQRT_HD).astype(np.float16),
            "bq": np.ascontiguousarray(
                (bq[sl] * INV_SQRT_HD).reshape(GH, HD).T).astype(np.float32),
            "wk": pack_w_heads(wk[:, sl]).astype(np.float16),
            "bk": np.ascontiguousarray(
                bk[sl].reshape(GH, HD).T).astype(np.float32),
            "wv": pack_w(wv[:, sl]).astype(np.float16),
            "bv": bv[sl][None, :].astype(np.float16),
            "ckT": pack_w(cache_k[b][:, sl].T).astype(np.float16),
            "cv": pack_w(cache_v[b][:, sl]).astype(np.float16),
            "wo": pack_w(wo_p[:, sl]).astype(np.float16),
            "bo": bo[sl][None, :].astype(np.float16),
        })
    return in_maps


def assemble(results):
    out = np.empty((B, S, D), np.float32)
    for c in range(8):
        b, g = divmod(c, 4)
        out[b, :, DG * g:DG * (g + 1)] = results[c]["y"]
    return out


def kernel(**inputs):
    nc = get_built()
    in_maps = make_in_maps(**inputs)
    res = run_bass_kernel_spmd(nc, in_maps, core_ids=list(range(8)))
    return assemble(res.results)
